# revision 6
# baseline (speedup 1.0000x reference)
"""Trainium2 Bass kernel for fused multi-head attention block.

Computes (per reference):
    q = query @ Wq ; k = key @ Wk ; v = value @ Wv        (B,S,D)
    per (b,h): A = softmax((q_h @ k_h^T) / sqrt(D)) ; o_h = A @ v_h
    o = merge_heads ; o = q + o
    out = o + relu(o @ Wo + bo)

Shapes: B=4, S=2048, D=1024, H=8 (head dim 128), fp32 in/out.

Sharding: 8 cores = (batch b in 0..3) x (query-seq half j in 0..1).
Each core computes 1024 query rows of one batch against that batch's
full K/V. No collectives. All device compute stays in a
feature-on-partition ("transposed") layout so no on-device transposes
are needed:
  - projections:   lhsT = weight tile (stationary), rhs = x^T     -> q^T, k^T
                   lhsT = xv^T tile,  rhs = Wv                    -> v natural
  - logits^T:      lhsT = k^T tile,   rhs = q^T                   -> [kv, q]
  - E = exp(logits^T / 32) on ScalarE (no max subtraction needed:
    logits/32 ~ N(0, 0.35), exp cannot overflow)
  - o^T:           lhsT = v natural,  rhs = E, accumulated over kv tiles
  - denom:         lhsT = ones[128,.,1], rhs = E  (M=1 matmuls)
  - z^T:           lhsT = Wo tile,    rhs = oq^T

Precision: q path (q projection, logits, z projection, residuals) runs
bf16 matmuls with fp32 PSUM; the attention-internal path (k/v
projections, exp tiles, A@V, denom) runs fp8e4m3 with DoubleRow perf
mode (2 contraction tiles per matmul) — softmax averages ~2048 values
so elementwise fp8 noise washes out in the output.

Schedule: single fused loop. Pass 1 (query chunk c=0) interleaves per
head: q/k projections, v projection tiles, attention. Pass 2 (c=1)
interleaves the c=0 output projection into the exp-bound attention
stream; the c=1 output projection is the only serial tail.
"""

import numpy as np
import ml_dtypes

BF16 = ml_dtypes.bfloat16
FP8 = ml_dtypes.float8_e4m3

FP8_ATTN = True     # exp tiles, A@V, denominator, vN storage in fp8 + DoubleRow
FP8_KVPROJ = True   # k/v projections from fp8 inputs with DoubleRow

B, S, D, H = 4, 2048, 1024, 8
DH = D // H          # 128
SH = S // 2          # 1024 query rows per core
NCORES = 8
KT = D // 128        # 8 contraction tiles
NKV = S // 128       # 16 kv tiles
QC = SH // 512       # 2 query chunks of 512
KC = S // 512        # 4 kv chunks of 512
DC = D // 512        # 2 dim chunks of 512
SCALE = 1.0 / 32.0   # 1/sqrt(D)

_CACHE = {}


def _build_nc():
    import concourse.bacc as bacc
    import concourse.tile as tile
    import concourse.mybir as mybir

    dt = mybir.dt
    AF = mybir.ActivationFunctionType
    ALU = mybir.AluOpType
    DR = mybir.MatmulPerfMode.DoubleRow
    kv_dt = dt.float8e4 if FP8_KVPROJ else dt.bfloat16
    at_dt = dt.float8e4 if FP8_ATTN else dt.bfloat16

    nc = bacc.Bacc("TRN2", target_bir_lowering=False, debug=False)

    xqT = nc.dram_tensor("xqT", [D, SH], dt.bfloat16, kind="ExternalInput").ap()
    xkT = nc.dram_tensor("xkT", [D, S], kv_dt, kind="ExternalInput").ap()
    xvT = nc.dram_tensor("xvT", [D, S], kv_dt, kind="ExternalInput").ap()
    wq = nc.dram_tensor("wq", [D, D], dt.bfloat16, kind="ExternalInput").ap()
    wk = nc.dram_tensor("wk", [D, D], kv_dt, kind="ExternalInput").ap()
    wv = nc.dram_tensor("wv", [D, D], kv_dt, kind="ExternalInput").ap()
    wo = nc.dram_tensor("wo", [D, D], dt.bfloat16, kind="ExternalInput").ap()
    bo = nc.dram_tensor("bo", [128, KT], dt.float32, kind="ExternalInput").ap()
    outT = nc.dram_tensor("outT", [D, SH], dt.float32, kind="ExternalOutput").ap()

    with tile.TileContext(nc) as tc:
        with (
            tc.tile_pool(name="persist", bufs=1) as persist,
            tc.tile_pool(name="xw", bufs=1) as xw,
            tc.tile_pool(name="etp", bufs=8) as etp,
            tc.tile_pool(name="p2t", bufs=2) as p2t,
            tc.tile_pool(name="psproj", bufs=2, space="PSUM") as psproj,
            tc.tile_pool(name="ps_l", bufs=2, space="PSUM") as psl,
            tc.tile_pool(name="ps_o", bufs=1, space="PSUM") as pso,
            tc.tile_pool(name="ps_d", bufs=1, space="PSUM") as psd,
        ):
            qT = persist.tile([128, H, SH], dt.bfloat16)   # [dh, head, qtok]
            kT = persist.tile([128, H, S], dt.bfloat16)    # [dh, head, kvtok]
            vN = persist.tile([128, NKV, D], at_dt)        # [kvtok%128, kvtile, d]
            oq_b = persist.tile([128, H, SH], dt.bfloat16)
            wo_sb = persist.tile([128, KT, D], dt.bfloat16)
            bo_sb = persist.tile([128, KT], dt.float32)
            ones_sb = persist.tile([128, 2, 16], at_dt)
            nc.gpsimd.memset(ones_sb[:], 1.0)

            xq_sb = xw.tile([128, KT, SH], dt.bfloat16)
            xk_sb = xw.tile([128, KT, S], kv_dt)
            xv_sb = xw.tile([128, KT, S], kv_dt)
            wq_sb = xw.tile([128, KT, D], dt.bfloat16)
            wk_sb = xw.tile([128, KT, D], kv_dt)
            wv_sb = xw.tile([128, KT, D], kv_dt)
            # DMA issue order matches first consumption: q, k weights/inputs
            # (head 0 projections), then v, then the late-used Wo / bo.
            for k in range(KT):
                r = slice(k * 128, (k + 1) * 128)
                nc.sync.dma_start(wq_sb[:, k, :], wq[r, :])
                nc.sync.dma_start(xq_sb[:, k, :], xqT[r, :])
            for k in range(KT):
                r = slice(k * 128, (k + 1) * 128)
                nc.sync.dma_start(wk_sb[:, k, :], wk[r, :])
                nc.sync.dma_start(xk_sb[:, k, :], xkT[r, :])
            for k in range(KT):
                r = slice(k * 128, (k + 1) * 128)
                nc.sync.dma_start(wv_sb[:, k, :], wv[r, :])
                nc.sync.dma_start(xv_sb[:, k, :], xvT[r, :])
            for k in range(KT):
                nc.sync.dma_start(wo_sb[:, k, :], wo[k * 128 : (k + 1) * 128, :])
            nc.sync.dma_start(bo_sb[:], bo[:])

            def q_proj(h):
                hs = slice(h * 128, (h + 1) * 128)
                for cc in range(QC):
                    cs = slice(cc * 512, (cc + 1) * 512)
                    ps = psproj.tile([128, 512], dt.float32, tag="pp", name="pp")
                    for k in range(KT):
                        nc.tensor.matmul(
                            ps[:], wq_sb[:, k, hs], xq_sb[:, k, cs],
                            start=(k == 0), stop=(k == KT - 1),
                        )
                    nc.vector.tensor_copy(qT[:, h, cs], ps[:])

            def k_proj(h):
                hs = slice(h * 128, (h + 1) * 128)
                for cc in range(KC):
                    cs = slice(cc * 512, (cc + 1) * 512)
                    ps = psproj.tile([128, 512], dt.float32, tag="pp", name="pp")
                    if FP8_KVPROJ:
                        for k in range(KT // 2):
                            nc.tensor.matmul(
                                ps[:],
                                wk_sb[:, 2 * k : 2 * k + 2, hs],
                                xk_sb[:, 2 * k : 2 * k + 2, cs],
                                start=(k == 0), stop=(k == KT // 2 - 1),
                                perf_mode=DR,
                            )
                    else:
                        for k in range(KT):
                            nc.tensor.matmul(
                                ps[:], wk_sb[:, k, hs], xk_sb[:, k, cs],
                                start=(k == 0), stop=(k == KT - 1),
                            )
                    nc.vector.tensor_copy(kT[:, h, cs], ps[:])

            def v_proj(t, dc):
                ts_ = slice(t * 128, (t + 1) * 128)
                cs = slice(dc * 512, (dc + 1) * 512)
                ps = psproj.tile([128, 512], dt.float32, tag="pp", name="pp")
                if FP8_KVPROJ:
                    for k in range(KT // 2):
                        nc.tensor.matmul(
                            ps[:],
                            xv_sb[:, 2 * k : 2 * k + 2, ts_],
                            wv_sb[:, 2 * k : 2 * k + 2, cs],
                            start=(k == 0), stop=(k == KT // 2 - 1),
                            perf_mode=DR,
                        )
                else:
                    for k in range(KT):
                        nc.tensor.matmul(
                            ps[:], xv_sb[:, k, ts_], wv_sb[:, k, cs],
                            start=(k == 0), stop=(k == KT - 1),
                        )
                nc.vector.tensor_copy(vN[:, t, cs], ps[:])

            def attn(h, c):
                hs = slice(h * 128, (h + 1) * 128)
                cs = slice(c * 512, (c + 1) * 512)
                ets = []
                for tp in range(NKV // 2):
                    # two logits matmuls into one 2-bank PSUM tile, then a
                    # single wide exp on ScalarE
                    pl = psl.tile([128, 1024], dt.float32, tag="pl", name="pl")
                    for u in range(2):
                        t = 2 * tp + u
                        ts_ = slice(t * 128, (t + 1) * 128)
                        nc.tensor.matmul(
                            pl[:, u * 512 : (u + 1) * 512],
                            kT[:, h, ts_], qT[:, h, cs],
                        )
                    et = etp.tile([128, 1024], at_dt, tag="et", name="et")
                    nc.scalar.activation(et[:], pl[:], AF.Exp, scale=SCALE)
                    ets.append(et)
                po = pso.tile([128, 512], dt.float32, tag="po", name="po")
                pd = psd.tile([1, 512], dt.float32, tag="pd", name="pd")
                if FP8_ATTN:
                    for tp in range(NKV // 2):
                        pair = ets[tp][:].rearrange("p (u n) -> p u n", u=2)
                        nc.tensor.matmul(
                            po[:], vN[:, 2 * tp : 2 * tp + 2, hs], pair,
                            start=(tp == 0), stop=(tp == NKV // 2 - 1),
                            perf_mode=DR,
                        )
                    for tp in range(NKV // 2):
                        pair = ets[tp][:].rearrange("p (u n) -> p u n", u=2)
                        nc.tensor.matmul(
                            pd[:], ones_sb[:, :, 0:1], pair,
                            start=(tp == 0), stop=(tp == NKV // 2 - 1),
                            perf_mode=DR,
                        )
                else:
                    for tp in range(NKV // 2):
                        for u in range(2):
                            t = 2 * tp + u
                            us = slice(u * 512, (u + 1) * 512)
                            nc.tensor.matmul(
                                po[:], vN[:, t, hs], ets[tp][:, us],
                                start=(t == 0), stop=(t == NKV - 1),
                            )
                    for tp in range(NKV // 2):
                        for u in range(2):
                            t = 2 * tp + u
                            us = slice(u * 512, (u + 1) * 512)
                            nc.tensor.matmul(
                                pd[:], ones_sb[:, 0, 0:1], ets[tp][:, us],
                                start=(t == 0), stop=(t == NKV - 1),
                            )
                dn = p2t.tile([1, 512], dt.float32, tag="dn", name="dn")
                nc.vector.tensor_copy(dn[:], pd[:])
                rb = p2t.tile([128, 512], dt.float32, tag="rb", name="rb")
                nc.gpsimd.partition_broadcast(rb[:], dn[:])
                nc.vector.reciprocal(rb[:], rb[:])
                on = p2t.tile([128, 512], dt.float32, tag="on", name="on")
                nc.vector.tensor_mul(on[:], po[:], rb[:])
                nc.vector.tensor_add(oq_b[:, h, cs], on[:], qT[:, h, cs])

            def z_group(nt, c):
                ns = slice(nt * 128, (nt + 1) * 128)
                cs = slice(c * 512, (c + 1) * 512)
                pz = psproj.tile([128, 512], dt.float32, tag="pp", name="pp")
                for k in range(KT):
                    nc.tensor.matmul(
                        pz[:], wo_sb[:, k, ns], oq_b[:, k, cs],
                        start=(k == 0), stop=(k == KT - 1),
                    )
                rel = p2t.tile([128, 512], dt.float32, tag="rel", name="rel")
                nc.vector.tensor_scalar(
                    rel[:], pz[:], bo_sb[:, nt : nt + 1], 0.0, ALU.add, ALU.max
                )
                ot = p2t.tile([128, 512], dt.float32, tag="ot", name="ot")
                nc.vector.tensor_add(ot[:], rel[:], oq_b[:, nt, cs])
                nc.sync.dma_start(outT[ns, cs], ot[:])

            # ---- pass 1: projections + attention for query chunk c=0
            for h in range(H):
                q_proj(h)
                k_proj(h)
                if h == 0:
                    for t in range(NKV):
                        v_proj(t, 0)
                attn(h, 0)
                if h < 4:
                    for t in range(4 * h, 4 * h + 4):
                        v_proj(t, 1)
            # ---- pass 2: attention c=1 with c=0 output projection interleaved
            for h in range(H):
                attn(h, 1)
                z_group(h, 0)
            # ---- tail: c=1 output projection
            for nt in range(KT):
                z_group(nt, 1)

    nc.compile()
    return nc


def _make_runner(nc, n_cores=NCORES):
    import jax
    from jax.experimental.shard_map import shard_map
    from jax.sharding import Mesh, PartitionSpec
    from concourse import bass2jax
    import concourse.mybir as mybir

    bass2jax.install_neuronx_cc_hook()
    assert nc.dbg_addr is None

    partition_name = nc.partition_id_tensor.name if nc.partition_id_tensor else None
    in_names, out_names, out_avals, zero_outs = [], [], [], []
    for alloc in nc.m.functions[0].allocations:
        if not isinstance(alloc, mybir.MemoryLocationSet):
            continue
        name = alloc.memorylocations[0].name
        if alloc.kind == "ExternalInput":
            if name != partition_name:
                in_names.append(name)
        elif alloc.kind == "ExternalOutput":
            shape = tuple(alloc.tensor_shape)
            dtype = mybir.dt.np(alloc.dtype)
            out_names.append(name)
            out_avals.append(jax.core.ShapedArray(shape, dtype))
            zero_outs.append(np.zeros(shape, dtype))
    n_params = len(in_names)
    n_outs = len(out_avals)
    in_names_all = list(in_names) + list(out_names)
    if partition_name is not None:
        in_names_all.append(partition_name)
    donate = tuple(range(n_params, n_params + n_outs))

    def _body(*args):
        operands = list(args)
        if partition_name is not None:
            operands.append(bass2jax.partition_id_tensor())
        outs = bass2jax._bass_exec_p.bind(
            *operands,
            out_avals=tuple(out_avals),
            in_names=tuple(in_names_all),
            out_names=tuple(out_names),
            lowering_input_output_aliases=(),
            sim_require_finite=True,
            sim_require_nnan=True,
            nc=nc,
        )
        return tuple(outs)

    devices = jax.devices()[:n_cores]
    assert len(devices) == n_cores
    mesh = Mesh(np.asarray(devices), ("core",))
    in_specs = (PartitionSpec("core"),) * (n_params + n_outs)
    out_specs = (PartitionSpec("core"),) * n_outs
    sharded = jax.jit(
        shard_map(
            _body, mesh=mesh, in_specs=in_specs, out_specs=out_specs, check_rep=False
        ),
        donate_argnums=donate,
        keep_unused=True,
    )

    class Runner:
        def __init__(self):
            self.mesh = mesh
            self.in_names = in_names
            self.out_names = out_names
            self.zero_outs = zero_outs
            self.n_cores = n_cores
            self.sharded = sharded

        def concat_inputs(self, in_maps):
            per_core = [[np.asarray(m[nm]) for nm in in_names] for m in in_maps]
            return [
                np.concatenate([per_core[c][i] for c in range(n_cores)], axis=0)
                for i in range(n_params)
            ]

        def concat_zeros(self):
            return [np.concatenate([z] * n_cores, axis=0) for z in zero_outs]

        def __call__(self, in_maps):
            concat_in = self.concat_inputs(in_maps)
            out_arrs = sharded(*concat_in, *self.concat_zeros())
            results = []
            for c in range(n_cores):
                res = {}
                for i, name in enumerate(out_names):
                    arr = np.asarray(out_arrs[i])
                    rows = arr.shape[0] // n_cores
                    res[name] = arr[c * rows : (c + 1) * rows]
                results.append(res)
            return results

    return Runner()


def _get_runner():
    if "runner" not in _CACHE:
        nc = _build_nc()
        _CACHE["runner"] = _make_runner(nc)
    return _CACHE["runner"]


def _prep_in_maps(query, key, value, Wq, Wk, Wv, Wo, bo):
    kv_np = FP8 if FP8_KVPROJ else BF16
    query = np.asarray(query, np.float32)
    key = np.asarray(key, np.float32)
    value = np.asarray(value, np.float32)
    wq_b = np.asarray(Wq, np.float32).astype(BF16)
    wk_b = np.asarray(Wk, np.float32).astype(kv_np)
    wv_b = np.asarray(Wv, np.float32).astype(kv_np)
    wo_b = np.asarray(Wo, np.float32).astype(BF16)
    bo_f = np.ascontiguousarray(np.asarray(bo, np.float32).reshape(KT, 128).T)
    in_maps = []
    for core in range(NCORES):
        b, jj = divmod(core, 2)
        rows = slice(jj * SH, (jj + 1) * SH)
        in_maps.append(
            dict(
                xqT=query[b, rows, :].T.astype(BF16),
                xkT=key[b].T.astype(kv_np),
                xvT=value[b].T.astype(kv_np),
                wq=wq_b, wk=wk_b, wv=wv_b, wo=wo_b, bo=bo_f,
            )
        )
    return in_maps


def kernel(query, key, value, Wq, Wk, Wv, Wo, bo):
    runner = _get_runner()
    in_maps = _prep_in_maps(query, key, value, Wq, Wk, Wv, Wo, bo)
    results = runner(in_maps)
    out = np.empty((B, S, D), np.float32)
    for core in range(NCORES):
        b, jj = divmod(core, 2)
        out[b, jj * SH : (jj + 1) * SH, :] = results[core]["outT"].T
    return out


# revision 10
# speedup vs baseline: 4.7152x; 4.7152x over previous
"""Trainium2 Bass kernel for fused multi-head attention block.

Computes (per reference):
    q = query @ Wq ; k = key @ Wk ; v = value @ Wv        (B,S,D)
    per (b,h): A = softmax((q_h @ k_h^T) / sqrt(D)) ; o_h = A @ v_h
    o = merge_heads ; o = q + o
    out = o + relu(o @ Wo + bo)

Shapes: B=4, S=2048, D=1024, H=8 (head dim 128), fp32 in/out.

Sharding: 8 cores = (batch b in 0..3) x (query-seq half j in 0..1).
Each core computes 1024 query rows of one batch against that batch's
full K/V. No collectives. All device compute stays in a
feature-on-partition ("transposed") layout so no on-device transposes
are needed:
  - projections:   lhsT = weight tile (stationary), rhs = x^T     -> q^T, k^T
                   lhsT = xv^T tile,  rhs = Wv                    -> v natural
  - logits^T:      lhsT = k^T tile,   rhs = q^T                   -> [kv, q]
  - E = exp(logits^T / 32) on ScalarE (no max subtraction needed:
    logits/32 ~ N(0, 0.35), exp cannot overflow)
  - o^T:           lhsT = v natural,  rhs = E, accumulated over kv tiles
  - denom:         lhsT = ones[128,.,1], rhs = E  (M=1 matmuls)
  - z^T:           lhsT = Wo tile,    rhs = oq^T

Precision: q path (q projection, logits, z projection, residuals) runs
bf16 matmuls with fp32 PSUM; the attention-internal path (k/v
projections, exp tiles, A@V, denom) runs fp8e4m3 with DoubleRow perf
mode (2 contraction tiles per matmul) — softmax averages ~2048 values
so elementwise fp8 noise washes out in the output.

Schedule: single fused loop. Pass 1 (query chunk c=0) interleaves per
head: q/k projections, v projection tiles, attention. Pass 2 (c=1)
interleaves the c=0 output projection into the exp-bound attention
stream; the c=1 output projection is the only serial tail.
"""

import numpy as np
import ml_dtypes

BF16 = ml_dtypes.bfloat16
FP8 = ml_dtypes.float8_e4m3

FP8_ATTN = True     # exp tiles, A@V, denominator, vN storage in fp8 + DoubleRow
FP8_KVPROJ = True   # k/v projections from fp8 inputs with DoubleRow

B, S, D, H = 4, 2048, 1024, 8
DH = D // H          # 128
SH = S // 2          # 1024 query rows per core
NCORES = 8
KT = D // 128        # 8 contraction tiles
NKV = S // 128       # 16 kv tiles
QC = SH // 512       # 2 query chunks of 512
KC = S // 512        # 4 kv chunks of 512
DC = D // 512        # 2 dim chunks of 512
SCALE = 1.0 / 32.0   # 1/sqrt(D)

_CACHE = {}


def _build_nc():
    import concourse.bacc as bacc
    import concourse.tile as tile
    import concourse.mybir as mybir

    dt = mybir.dt
    AF = mybir.ActivationFunctionType
    ALU = mybir.AluOpType
    DR = mybir.MatmulPerfMode.DoubleRow
    kv_dt = dt.float8e4 if FP8_KVPROJ else dt.bfloat16
    at_dt = dt.float8e4 if FP8_ATTN else dt.bfloat16

    nc = bacc.Bacc("TRN2", target_bir_lowering=False, debug=False)

    xqT = nc.dram_tensor("xqT", [D, SH], dt.bfloat16, kind="ExternalInput").ap()
    xkT = nc.dram_tensor("xkT", [D, S], kv_dt, kind="ExternalInput").ap()
    xvT = nc.dram_tensor("xvT", [D, S], kv_dt, kind="ExternalInput").ap()
    wq = nc.dram_tensor("wq", [D, D], dt.bfloat16, kind="ExternalInput").ap()
    wk = nc.dram_tensor("wk", [D, D], kv_dt, kind="ExternalInput").ap()
    wv = nc.dram_tensor("wv", [D, D], kv_dt, kind="ExternalInput").ap()
    wo = nc.dram_tensor("wo", [D, D], dt.bfloat16, kind="ExternalInput").ap()
    bo = nc.dram_tensor("bo", [128, KT], dt.float32, kind="ExternalInput").ap()
    outT = nc.dram_tensor("outT", [D, SH], dt.float32, kind="ExternalOutput").ap()

    with tile.TileContext(nc) as tc:
        with (
            tc.tile_pool(name="persist", bufs=1) as persist,
            tc.tile_pool(name="xw", bufs=1) as xw,
            tc.tile_pool(name="etp", bufs=8) as etp,
            tc.tile_pool(name="p2t", bufs=2) as p2t,
            tc.tile_pool(name="psproj", bufs=2, space="PSUM") as psproj,
            tc.tile_pool(name="ps_l", bufs=2, space="PSUM") as psl,
            tc.tile_pool(name="ps_o", bufs=1, space="PSUM") as pso,
            tc.tile_pool(name="ps_d", bufs=1, space="PSUM") as psd,
        ):
            qT = persist.tile([128, H, SH], dt.bfloat16)   # [dh, head, qtok]
            kT = persist.tile([128, H, S], dt.bfloat16)    # [dh, head, kvtok]
            vN = persist.tile([128, NKV, D], at_dt)        # [kvtok%128, kvtile, d]
            oq_b = persist.tile([128, H, SH], dt.bfloat16)
            wo_sb = persist.tile([128, KT, D], dt.bfloat16)
            bo_sb = persist.tile([128, KT], dt.float32)
            ones_sb = persist.tile([128, 2, 16], at_dt)
            nc.gpsimd.memset(ones_sb[:], 1.0)

            xq_sb = xw.tile([128, KT, SH], dt.bfloat16)
            xk_sb = xw.tile([128, KT, S], kv_dt)
            xv_sb = xw.tile([128, KT, S], kv_dt)
            wq_sb = xw.tile([128, KT, D], dt.bfloat16)
            wk_sb = xw.tile([128, KT, D], kv_dt)
            wv_sb = xw.tile([128, KT, D], kv_dt)
            # DMA issue order matches first consumption: q, k weights/inputs
            # (head 0 projections), then v, then the late-used Wo / bo.
            for k in range(KT):
                r = slice(k * 128, (k + 1) * 128)
                nc.sync.dma_start(wq_sb[:, k, :], wq[r, :])
                nc.sync.dma_start(xq_sb[:, k, :], xqT[r, :])
            for k in range(KT):
                r = slice(k * 128, (k + 1) * 128)
                nc.sync.dma_start(wk_sb[:, k, :], wk[r, :])
                nc.sync.dma_start(xk_sb[:, k, :], xkT[r, :])
            for k in range(KT):
                r = slice(k * 128, (k + 1) * 128)
                nc.sync.dma_start(wv_sb[:, k, :], wv[r, :])
                nc.sync.dma_start(xv_sb[:, k, :], xvT[r, :])
            for k in range(KT):
                nc.sync.dma_start(wo_sb[:, k, :], wo[k * 128 : (k + 1) * 128, :])
            nc.sync.dma_start(bo_sb[:], bo[:])

            def q_proj(h):
                hs = slice(h * 128, (h + 1) * 128)
                for cc in range(QC):
                    cs = slice(cc * 512, (cc + 1) * 512)
                    ps = psproj.tile([128, 512], dt.float32, tag="pp", name="pp")
                    for k in range(KT):
                        nc.tensor.matmul(
                            ps[:], wq_sb[:, k, hs], xq_sb[:, k, cs],
                            start=(k == 0), stop=(k == KT - 1),
                        )
                    nc.vector.tensor_copy(qT[:, h, cs], ps[:])

            def k_proj(h):
                hs = slice(h * 128, (h + 1) * 128)
                for cc in range(KC):
                    cs = slice(cc * 512, (cc + 1) * 512)
                    ps = psproj.tile([128, 512], dt.float32, tag="pp", name="pp")
                    if FP8_KVPROJ:
                        for k in range(KT // 2):
                            nc.tensor.matmul(
                                ps[:],
                                wk_sb[:, 2 * k : 2 * k + 2, hs],
                                xk_sb[:, 2 * k : 2 * k + 2, cs],
                                start=(k == 0), stop=(k == KT // 2 - 1),
                                perf_mode=DR,
                            )
                    else:
                        for k in range(KT):
                            nc.tensor.matmul(
                                ps[:], wk_sb[:, k, hs], xk_sb[:, k, cs],
                                start=(k == 0), stop=(k == KT - 1),
                            )
                    nc.vector.tensor_copy(kT[:, h, cs], ps[:])

            def v_proj(t, dc):
                ts_ = slice(t * 128, (t + 1) * 128)
                cs = slice(dc * 512, (dc + 1) * 512)
                ps = psproj.tile([128, 512], dt.float32, tag="pp", name="pp")
                if FP8_KVPROJ:
                    for k in range(KT // 2):
                        nc.tensor.matmul(
                            ps[:],
                            xv_sb[:, 2 * k : 2 * k + 2, ts_],
                            wv_sb[:, 2 * k : 2 * k + 2, cs],
                            start=(k == 0), stop=(k == KT // 2 - 1),
                            perf_mode=DR,
                        )
                else:
                    for k in range(KT):
                        nc.tensor.matmul(
                            ps[:], xv_sb[:, k, ts_], wv_sb[:, k, cs],
                            start=(k == 0), stop=(k == KT - 1),
                        )
                nc.vector.tensor_copy(vN[:, t, cs], ps[:])

            def attn(h, c):
                hs = slice(h * 128, (h + 1) * 128)
                cs = slice(c * 512, (c + 1) * 512)
                ets = []
                for tp in range(NKV // 2):
                    # two logits matmuls into one 2-bank PSUM tile, then a
                    # single wide exp on ScalarE
                    pl = psl.tile([128, 1024], dt.float32, tag="pl", name="pl")
                    for u in range(2):
                        t = 2 * tp + u
                        ts_ = slice(t * 128, (t + 1) * 128)
                        nc.tensor.matmul(
                            pl[:, u * 512 : (u + 1) * 512],
                            kT[:, h, ts_], qT[:, h, cs],
                        )
                    et = etp.tile([128, 1024], at_dt, tag="et", name="et")
                    nc.scalar.activation(et[:], pl[:], AF.Exp, scale=SCALE)
                    ets.append(et)
                po = pso.tile([128, 512], dt.float32, tag="po", name="po")
                pd = psd.tile([1, 512], dt.float32, tag="pd", name="pd")
                if FP8_ATTN:
                    for tp in range(NKV // 2):
                        pair = ets[tp][:].rearrange("p (u n) -> p u n", u=2)
                        nc.tensor.matmul(
                            po[:], vN[:, 2 * tp : 2 * tp + 2, hs], pair,
                            start=(tp == 0), stop=(tp == NKV // 2 - 1),
                            perf_mode=DR,
                        )
                    for tp in range(NKV // 2):
                        pair = ets[tp][:].rearrange("p (u n) -> p u n", u=2)
                        nc.tensor.matmul(
                            pd[:], ones_sb[:, :, 0:1], pair,
                            start=(tp == 0), stop=(tp == NKV // 2 - 1),
                            perf_mode=DR,
                        )
                else:
                    for tp in range(NKV // 2):
                        for u in range(2):
                            t = 2 * tp + u
                            us = slice(u * 512, (u + 1) * 512)
                            nc.tensor.matmul(
                                po[:], vN[:, t, hs], ets[tp][:, us],
                                start=(t == 0), stop=(t == NKV - 1),
                            )
                    for tp in range(NKV // 2):
                        for u in range(2):
                            t = 2 * tp + u
                            us = slice(u * 512, (u + 1) * 512)
                            nc.tensor.matmul(
                                pd[:], ones_sb[:, 0, 0:1], ets[tp][:, us],
                                start=(t == 0), stop=(t == NKV - 1),
                            )
                dn = p2t.tile([1, 512], dt.float32, tag="dn", name="dn")
                nc.vector.tensor_copy(dn[:], pd[:])
                rb = p2t.tile([128, 512], dt.float32, tag="rb", name="rb")
                nc.gpsimd.partition_broadcast(rb[:], dn[:])
                nc.vector.reciprocal(rb[:], rb[:])
                on = p2t.tile([128, 512], dt.float32, tag="on", name="on")
                nc.vector.tensor_mul(on[:], po[:], rb[:])
                nc.vector.tensor_add(oq_b[:, h, cs], on[:], qT[:, h, cs])

            def z_group(nt, c):
                ns = slice(nt * 128, (nt + 1) * 128)
                cs = slice(c * 512, (c + 1) * 512)
                pz = psproj.tile([128, 512], dt.float32, tag="pp", name="pp")
                for k in range(KT):
                    nc.tensor.matmul(
                        pz[:], wo_sb[:, k, ns], oq_b[:, k, cs],
                        start=(k == 0), stop=(k == KT - 1),
                    )
                rel = p2t.tile([128, 512], dt.float32, tag="rel", name="rel")
                nc.vector.tensor_scalar(
                    rel[:], pz[:], bo_sb[:, nt : nt + 1], 0.0, ALU.add, ALU.max
                )
                ot = p2t.tile([128, 512], dt.float32, tag="ot", name="ot")
                nc.vector.tensor_add(ot[:], rel[:], oq_b[:, nt, cs])
                nc.sync.dma_start(outT[ns, cs], ot[:])

            # ---- pass 1: projections + attention for query chunk c=0
            for h in range(H):
                q_proj(h)
                k_proj(h)
                if h == 0:
                    for t in range(NKV):
                        v_proj(t, 0)
                attn(h, 0)
                if h < 4:
                    for t in range(4 * h, 4 * h + 4):
                        v_proj(t, 1)
            # ---- pass 2: attention c=1 with c=0 output projection interleaved
            for h in range(H):
                attn(h, 1)
                z_group(h, 0)
            # ---- tail: c=1 output projection
            for nt in range(KT):
                z_group(nt, 1)

    nc.compile()
    return nc


def _make_runner(nc, n_cores=NCORES):
    import jax
    from jax.experimental.shard_map import shard_map
    from jax.sharding import Mesh, PartitionSpec
    from concourse import bass2jax
    import concourse.mybir as mybir

    bass2jax.install_neuronx_cc_hook()
    assert nc.dbg_addr is None

    partition_name = nc.partition_id_tensor.name if nc.partition_id_tensor else None
    in_names, out_names, out_avals, zero_outs = [], [], [], []
    for alloc in nc.m.functions[0].allocations:
        if not isinstance(alloc, mybir.MemoryLocationSet):
            continue
        name = alloc.memorylocations[0].name
        if alloc.kind == "ExternalInput":
            if name != partition_name:
                in_names.append(name)
        elif alloc.kind == "ExternalOutput":
            shape = tuple(alloc.tensor_shape)
            dtype = mybir.dt.np(alloc.dtype)
            out_names.append(name)
            out_avals.append(jax.core.ShapedArray(shape, dtype))
            zero_outs.append(np.zeros(shape, dtype))
    n_params = len(in_names)
    n_outs = len(out_avals)
    in_names_all = list(in_names) + list(out_names)
    if partition_name is not None:
        in_names_all.append(partition_name)
    donate = tuple(range(n_params, n_params + n_outs))

    def _body(*args):
        operands = list(args)
        if partition_name is not None:
            operands.append(bass2jax.partition_id_tensor())
        outs = bass2jax._bass_exec_p.bind(
            *operands,
            out_avals=tuple(out_avals),
            in_names=tuple(in_names_all),
            out_names=tuple(out_names),
            lowering_input_output_aliases=(),
            sim_require_finite=True,
            sim_require_nnan=True,
            nc=nc,
        )
        return tuple(outs)

    devices = jax.devices()[:n_cores]
    assert len(devices) == n_cores
    mesh = Mesh(np.asarray(devices), ("core",))
    in_specs = (PartitionSpec("core"),) * (n_params + n_outs)
    out_specs = (PartitionSpec("core"),) * n_outs
    sharded = jax.jit(
        shard_map(
            _body, mesh=mesh, in_specs=in_specs, out_specs=out_specs, check_rep=False
        ),
        donate_argnums=donate,
        keep_unused=True,
    )

    class Runner:
        def __init__(self):
            self.mesh = mesh
            self.in_names = in_names
            self.out_names = out_names
            self.zero_outs = zero_outs
            self.n_cores = n_cores
            self.sharded = sharded

        def concat_inputs(self, in_maps):
            per_core = [[np.asarray(m[nm]) for nm in in_names] for m in in_maps]
            return [
                np.concatenate([per_core[c][i] for c in range(n_cores)], axis=0)
                for i in range(n_params)
            ]

        def concat_zeros(self):
            return [np.concatenate([z] * n_cores, axis=0) for z in zero_outs]

        def __call__(self, in_maps):
            concat_in = self.concat_inputs(in_maps)
            out_arrs = sharded(*concat_in, *self.concat_zeros())
            results = []
            for c in range(n_cores):
                res = {}
                for i, name in enumerate(out_names):
                    arr = np.asarray(out_arrs[i])
                    rows = arr.shape[0] // n_cores
                    res[name] = arr[c * rows : (c + 1) * rows]
                results.append(res)
            return results

    return Runner()


def _get_runner():
    if "runner" not in _CACHE:
        nc = _build_nc()
        _CACHE["runner"] = _make_runner(nc)
    return _CACHE["runner"]


def _prep_in_maps(query, key, value, Wq, Wk, Wv, Wo, bo):
    kv_np = FP8 if FP8_KVPROJ else BF16
    query = np.asarray(query, np.float32)
    key = np.asarray(key, np.float32)
    value = np.asarray(value, np.float32)
    wq_b = np.asarray(Wq, np.float32).astype(BF16)
    wk_b = np.asarray(Wk, np.float32).astype(kv_np)
    wv_b = np.asarray(Wv, np.float32).astype(kv_np)
    wo_b = np.asarray(Wo, np.float32).astype(BF16)
    bo_f = np.ascontiguousarray(np.asarray(bo, np.float32).reshape(KT, 128).T)
    in_maps = []
    for core in range(NCORES):
        b, jj = divmod(core, 2)
        rows = slice(jj * SH, (jj + 1) * SH)
        in_maps.append(
            dict(
                xqT=query[b, rows, :].T.astype(BF16),
                xkT=key[b].T.astype(kv_np),
                xvT=value[b].T.astype(kv_np),
                wq=wq_b, wk=wk_b, wv=wv_b, wo=wo_b, bo=bo_f,
            )
        )
    return in_maps


def kernel(query, key, value, Wq, Wk, Wv, Wo, bo):
    runner = _get_runner()
    in_maps = _prep_in_maps(query, key, value, Wq, Wk, Wv, Wo, bo)
    results = runner(in_maps)
    out = np.empty((B, S, D), np.float32)
    for core in range(NCORES):
        b, jj = divmod(core, 2)
        out[b, jj * SH : (jj + 1) * SH, :] = results[core]["outT"].T
    return out


# revision 15
# speedup vs baseline: 6.3732x; 1.3516x over previous
"""Trainium2 Bass kernel for fused multi-head attention block.

Computes (per reference):
    q = query @ Wq ; k = key @ Wk ; v = value @ Wv        (B,S,D)
    per (b,h): A = softmax((q_h @ k_h^T) / sqrt(D)) ; o_h = A @ v_h
    o = merge_heads ; o = q + o
    out = o + relu(o @ Wo + bo)

Shapes: B=4, S=2048, D=1024, H=8 (head dim 128), fp32 in/out.

Sharding: 8 cores = (batch b in 0..3) x (query-seq half j in 0..1).
Each core computes 1024 query rows of one batch against that batch's
full K/V. No collectives. All device compute stays in a
feature-on-partition ("transposed") layout so no on-device transposes
are needed:
  - projections:   lhsT = weight tile (stationary), rhs = x^T     -> q^T, k^T
                   lhsT = xv^T tile,  rhs = Wv                    -> v natural
  - logits^T:      lhsT = k^T tile,   rhs = q^T                   -> [kv, q]
  - E = exp(logits^T / 32) on ScalarE (no max subtraction needed:
    logits/32 ~ N(0, 0.35), exp cannot overflow)
  - o^T:           lhsT = v natural,  rhs = E, accumulated over kv tiles
  - denom:         lhsT = ones[128,.,1], rhs = E  (M=1 matmuls)
  - z^T:           lhsT = Wo tile,    rhs = oq^T

Precision: q path (q projection, logits, z projection, residuals) runs
bf16 matmuls with fp32 PSUM; the attention-internal path (k/v
projections, exp tiles, A@V, denom) runs fp8e4m3 with DoubleRow perf
mode (2 contraction tiles per matmul) — softmax averages ~2048 values
so elementwise fp8 noise washes out in the output.

Schedule: single fused loop. Pass 1 (query chunk c=0) interleaves per
head: q/k projections, v projection tiles, attention. Pass 2 (c=1)
interleaves the c=0 output projection into the exp-bound attention
stream; the c=1 output projection is the only serial tail.
"""

import numpy as np
import ml_dtypes

BF16 = ml_dtypes.bfloat16
FP8 = ml_dtypes.float8_e4m3

FP8_ATTN = True     # exp tiles, A@V, denominator, vN storage in fp8 + DoubleRow
FP8_KVPROJ = True   # k/v projections from fp8 inputs with DoubleRow

B, S, D, H = 4, 2048, 1024, 8
DH = D // H          # 128
SH = S // 2          # 1024 query rows per core
NCORES = 8
KT = D // 128        # 8 contraction tiles
NKV = S // 128       # 16 kv tiles
QC = SH // 512       # 2 query chunks of 512
KC = S // 512        # 4 kv chunks of 512
DC = D // 512        # 2 dim chunks of 512
SCALE = 1.0 / 32.0   # 1/sqrt(D)

_CACHE = {}


def _build_nc():
    import concourse.bacc as bacc
    import concourse.tile as tile
    import concourse.mybir as mybir

    dt = mybir.dt
    AF = mybir.ActivationFunctionType
    ALU = mybir.AluOpType
    DR = mybir.MatmulPerfMode.DoubleRow
    kv_dt = dt.float8e4 if FP8_KVPROJ else dt.bfloat16
    at_dt = dt.float8e4 if FP8_ATTN else dt.bfloat16

    nc = bacc.Bacc("TRN2", target_bir_lowering=False, debug=False)

    xqT = nc.dram_tensor("xqT", [D, SH], dt.bfloat16, kind="ExternalInput").ap()
    xkT = nc.dram_tensor("xkT", [D, S], kv_dt, kind="ExternalInput").ap()
    xvT = nc.dram_tensor("xvT", [D, S], kv_dt, kind="ExternalInput").ap()
    wq = nc.dram_tensor("wq", [D, D], dt.bfloat16, kind="ExternalInput").ap()
    wk = nc.dram_tensor("wk", [D, D], kv_dt, kind="ExternalInput").ap()
    wv = nc.dram_tensor("wv", [D, D], kv_dt, kind="ExternalInput").ap()
    wo = nc.dram_tensor("wo", [D, D], dt.bfloat16, kind="ExternalInput").ap()
    bo = nc.dram_tensor("bo", [128, KT], dt.float32, kind="ExternalInput").ap()
    outT = nc.dram_tensor("outT", [D, SH], dt.float32, kind="ExternalOutput").ap()

    with tile.TileContext(nc) as tc:
        with (
            tc.tile_pool(name="persist", bufs=1) as persist,
            tc.tile_pool(name="xw", bufs=1) as xw,
            tc.tile_pool(name="etp", bufs=8) as etp,
            tc.tile_pool(name="p2t", bufs=2) as p2t,
            tc.tile_pool(name="psproj", bufs=3, space="PSUM") as psproj,
            tc.tile_pool(name="ps_l", bufs=2, space="PSUM") as psl,
            tc.tile_pool(name="ps_o", bufs=1, space="PSUM") as pso,
        ):
            qT = persist.tile([128, H, SH], dt.bfloat16)   # [dh, head, qtok]
            kT = persist.tile([128, H, S], dt.bfloat16)    # [dh, head, kvtok]
            vN = persist.tile([128, NKV, D], at_dt)        # [kvtok%128, kvtile, d]
            oq_b = persist.tile([128, H, SH], dt.bfloat16)
            wo_sb = persist.tile([128, KT, D], dt.bfloat16)
            bo_sb = persist.tile([128, KT], dt.float32)
            ones_sb = persist.tile([128, 2, 16], at_dt)
            nc.gpsimd.memset(ones_sb[:], 1.0)

            xq_sb = xw.tile([128, KT, SH], dt.bfloat16)
            xk_sb = xw.tile([128, KT, S], kv_dt)
            xv_sb = xw.tile([128, KT, S], kv_dt)
            wq_sb = xw.tile([128, KT, D], dt.bfloat16)
            wk_sb = xw.tile([128, KT, D], kv_dt)
            wv_sb = xw.tile([128, KT, D], kv_dt)
            # DMA issue order matches first consumption: q, k weights/inputs
            # (head 0 projections), then v, then the late-used Wo / bo.
            for k in range(KT):
                r = slice(k * 128, (k + 1) * 128)
                nc.sync.dma_start(wq_sb[:, k, :], wq[r, :])
                nc.sync.dma_start(xq_sb[:, k, :], xqT[r, :])
            for k in range(KT):
                r = slice(k * 128, (k + 1) * 128)
                nc.sync.dma_start(wk_sb[:, k, :], wk[r, :])
                nc.sync.dma_start(xk_sb[:, k, :], xkT[r, :])
            for k in range(KT):
                r = slice(k * 128, (k + 1) * 128)
                nc.sync.dma_start(wv_sb[:, k, :], wv[r, :])
                nc.sync.dma_start(xv_sb[:, k, :], xvT[r, :])
            for k in range(KT):
                nc.sync.dma_start(wo_sb[:, k, :], wo[k * 128 : (k + 1) * 128, :])
            nc.sync.dma_start(bo_sb[:], bo[:])

            def q_proj(h):
                hs = slice(h * 128, (h + 1) * 128)
                for cc in range(QC):
                    cs = slice(cc * 512, (cc + 1) * 512)
                    ps = psproj.tile([128, 512], dt.float32, tag="pp", name="pp")
                    for k in range(KT):
                        nc.tensor.matmul(
                            ps[:], wq_sb[:, k, hs], xq_sb[:, k, cs],
                            start=(k == 0), stop=(k == KT - 1),
                        )
                    nc.scalar.copy(qT[:, h, cs], ps[:])

            def k_proj(h):
                hs = slice(h * 128, (h + 1) * 128)
                for cc in range(KC):
                    cs = slice(cc * 512, (cc + 1) * 512)
                    ps = psproj.tile([128, 512], dt.float32, tag="pp", name="pp")
                    if FP8_KVPROJ:
                        for k in range(KT // 2):
                            nc.tensor.matmul(
                                ps[:],
                                wk_sb[:, 2 * k : 2 * k + 2, hs],
                                xk_sb[:, 2 * k : 2 * k + 2, cs],
                                start=(k == 0), stop=(k == KT // 2 - 1),
                                perf_mode=DR,
                            )
                    else:
                        for k in range(KT):
                            nc.tensor.matmul(
                                ps[:], wk_sb[:, k, hs], xk_sb[:, k, cs],
                                start=(k == 0), stop=(k == KT - 1),
                            )
                    nc.vector.tensor_copy(kT[:, h, cs], ps[:])

            def v_proj(t, dc):
                ts_ = slice(t * 128, (t + 1) * 128)
                cs = slice(dc * 512, (dc + 1) * 512)
                ps = psproj.tile([128, 512], dt.float32, tag="pp", name="pp")
                if FP8_KVPROJ:
                    for k in range(KT // 2):
                        nc.tensor.matmul(
                            ps[:],
                            xv_sb[:, 2 * k : 2 * k + 2, ts_],
                            wv_sb[:, 2 * k : 2 * k + 2, cs],
                            start=(k == 0), stop=(k == KT // 2 - 1),
                            perf_mode=DR,
                        )
                else:
                    for k in range(KT):
                        nc.tensor.matmul(
                            ps[:], xv_sb[:, k, ts_], wv_sb[:, k, cs],
                            start=(k == 0), stop=(k == KT - 1),
                        )
                nc.vector.tensor_copy(vN[:, t, cs], ps[:])

            def attn(h, c):
                hs = slice(h * 128, (h + 1) * 128)
                cs = slice(c * 512, (c + 1) * 512)
                ets = []
                for tp in range(NKV // 2):
                    # two logits matmuls into one 2-bank PSUM tile, then a
                    # single wide exp on ScalarE
                    pl = psl.tile([128, 1024], dt.float32, tag="pl", name="pl")
                    for u in range(2):
                        t = 2 * tp + u
                        ts_ = slice(t * 128, (t + 1) * 128)
                        nc.tensor.matmul(
                            pl[:, u * 512 : (u + 1) * 512],
                            kT[:, h, ts_], qT[:, h, cs],
                        )
                    et = etp.tile([128, 1024], at_dt, tag="et", name="et")
                    nc.scalar.activation(et[:], pl[:], AF.Exp, scale=SCALE)
                    ets.append(et)
                po = pso.tile([128, 512], dt.float32, tag="po", name="po")
                pdt = psproj.tile([128, 512], dt.float32, tag="pp", name="pdt")
                pd = pdt[0:1, :]
                if FP8_ATTN:
                    for tp in range(NKV // 2):
                        pair = ets[tp][:].rearrange("p (u n) -> p u n", u=2)
                        nc.tensor.matmul(
                            po[:], vN[:, 2 * tp : 2 * tp + 2, hs], pair,
                            start=(tp == 0), stop=(tp == NKV // 2 - 1),
                            perf_mode=DR,
                        )
                    for tp in range(NKV // 2):
                        pair = ets[tp][:].rearrange("p (u n) -> p u n", u=2)
                        nc.tensor.matmul(
                            pd, ones_sb[:, :, 0:1], pair,
                            start=(tp == 0), stop=(tp == NKV // 2 - 1),
                            perf_mode=DR,
                        )
                else:
                    for tp in range(NKV // 2):
                        for u in range(2):
                            t = 2 * tp + u
                            us = slice(u * 512, (u + 1) * 512)
                            nc.tensor.matmul(
                                po[:], vN[:, t, hs], ets[tp][:, us],
                                start=(t == 0), stop=(t == NKV - 1),
                            )
                    for tp in range(NKV // 2):
                        for u in range(2):
                            t = 2 * tp + u
                            us = slice(u * 512, (u + 1) * 512)
                            nc.tensor.matmul(
                                pd, ones_sb[:, 0, 0:1], ets[tp][:, us],
                                start=(t == 0), stop=(t == NKV - 1),
                            )
                dn = p2t.tile([1, 512], dt.float32, tag="dn", name="dn")
                nc.vector.tensor_copy(dn[:], pd)
                rb = p2t.tile([128, 512], dt.float32, tag="rb", name="rb")
                nc.gpsimd.partition_broadcast(rb[:], dn[:])
                nc.vector.reciprocal(rb[:], rb[:])
                on = p2t.tile([128, 512], dt.float32, tag="on", name="on")
                nc.vector.tensor_mul(on[:], po[:], rb[:])
                nc.vector.tensor_add(oq_b[:, h, cs], on[:], qT[:, h, cs])

            def z_group(nt, c):
                ns = slice(nt * 128, (nt + 1) * 128)
                cs = slice(c * 512, (c + 1) * 512)
                pz = psproj.tile([128, 512], dt.float32, tag="pp", name="pp")
                for k in range(KT):
                    nc.tensor.matmul(
                        pz[:], wo_sb[:, k, ns], oq_b[:, k, cs],
                        start=(k == 0), stop=(k == KT - 1),
                    )
                rel = p2t.tile([128, 512], dt.float32, tag="rel", name="rel")
                nc.vector.tensor_scalar(
                    rel[:], pz[:], bo_sb[:, nt : nt + 1], 0.0, ALU.add, ALU.max
                )
                ot = p2t.tile([128, 512], dt.float32, tag="ot", name="ot")
                nc.vector.tensor_add(ot[:], rel[:], oq_b[:, nt, cs])
                nc.sync.dma_start(outT[ns, cs], ot[:])

            # ---- pass 1: projections + attention for query chunk c=0
            for h in range(H):
                q_proj(h)
                k_proj(h)
                if h == 0:
                    for t in range(NKV):
                        v_proj(t, 0)
                attn(h, 0)
                if h < 4:
                    for t in range(4 * h, 4 * h + 4):
                        v_proj(t, 1)
            # ---- pass 2: attention c=1 with c=0 output projection interleaved
            for h in range(H):
                attn(h, 1)
                z_group(h, 0)
            # ---- tail: c=1 output projection
            for nt in range(KT):
                z_group(nt, 1)

    nc.compile()
    return nc


def _make_runner(nc, n_cores=NCORES):
    import jax
    from jax.experimental.shard_map import shard_map
    from jax.sharding import Mesh, PartitionSpec
    from concourse import bass2jax
    import concourse.mybir as mybir

    bass2jax.install_neuronx_cc_hook()
    assert nc.dbg_addr is None

    partition_name = nc.partition_id_tensor.name if nc.partition_id_tensor else None
    in_names, out_names, out_avals, zero_outs = [], [], [], []
    for alloc in nc.m.functions[0].allocations:
        if not isinstance(alloc, mybir.MemoryLocationSet):
            continue
        name = alloc.memorylocations[0].name
        if alloc.kind == "ExternalInput":
            if name != partition_name:
                in_names.append(name)
        elif alloc.kind == "ExternalOutput":
            shape = tuple(alloc.tensor_shape)
            dtype = mybir.dt.np(alloc.dtype)
            out_names.append(name)
            out_avals.append(jax.core.ShapedArray(shape, dtype))
            zero_outs.append(np.zeros(shape, dtype))
    n_params = len(in_names)
    n_outs = len(out_avals)
    in_names_all = list(in_names) + list(out_names)
    if partition_name is not None:
        in_names_all.append(partition_name)
    donate = tuple(range(n_params, n_params + n_outs))

    def _body(*args):
        operands = list(args)
        if partition_name is not None:
            operands.append(bass2jax.partition_id_tensor())
        outs = bass2jax._bass_exec_p.bind(
            *operands,
            out_avals=tuple(out_avals),
            in_names=tuple(in_names_all),
            out_names=tuple(out_names),
            lowering_input_output_aliases=(),
            sim_require_finite=True,
            sim_require_nnan=True,
            nc=nc,
        )
        return tuple(outs)

    devices = jax.devices()[:n_cores]
    assert len(devices) == n_cores
    mesh = Mesh(np.asarray(devices), ("core",))
    in_specs = (PartitionSpec("core"),) * (n_params + n_outs)
    out_specs = (PartitionSpec("core"),) * n_outs
    sharded = jax.jit(
        shard_map(
            _body, mesh=mesh, in_specs=in_specs, out_specs=out_specs, check_rep=False
        ),
        donate_argnums=donate,
        keep_unused=True,
    )

    class Runner:
        def __init__(self):
            self.mesh = mesh
            self.in_names = in_names
            self.out_names = out_names
            self.zero_outs = zero_outs
            self.n_cores = n_cores
            self.sharded = sharded

        def concat_inputs(self, in_maps):
            per_core = [[np.asarray(m[nm]) for nm in in_names] for m in in_maps]
            return [
                np.concatenate([per_core[c][i] for c in range(n_cores)], axis=0)
                for i in range(n_params)
            ]

        def concat_zeros(self):
            return [np.concatenate([z] * n_cores, axis=0) for z in zero_outs]

        def __call__(self, in_maps):
            concat_in = self.concat_inputs(in_maps)
            out_arrs = sharded(*concat_in, *self.concat_zeros())
            results = []
            for c in range(n_cores):
                res = {}
                for i, name in enumerate(out_names):
                    arr = np.asarray(out_arrs[i])
                    rows = arr.shape[0] // n_cores
                    res[name] = arr[c * rows : (c + 1) * rows]
                results.append(res)
            return results

    return Runner()


def _get_runner():
    if "runner" not in _CACHE:
        nc = _build_nc()
        _CACHE["runner"] = _make_runner(nc)
    return _CACHE["runner"]


def _prep_in_maps(query, key, value, Wq, Wk, Wv, Wo, bo):
    kv_np = FP8 if FP8_KVPROJ else BF16
    query = np.asarray(query, np.float32)
    key = np.asarray(key, np.float32)
    value = np.asarray(value, np.float32)
    wq_b = np.asarray(Wq, np.float32).astype(BF16)
    wk_b = np.asarray(Wk, np.float32).astype(kv_np)
    wv_b = np.asarray(Wv, np.float32).astype(kv_np)
    wo_b = np.asarray(Wo, np.float32).astype(BF16)
    bo_f = np.ascontiguousarray(np.asarray(bo, np.float32).reshape(KT, 128).T)
    in_maps = []
    for core in range(NCORES):
        b, jj = divmod(core, 2)
        rows = slice(jj * SH, (jj + 1) * SH)
        in_maps.append(
            dict(
                xqT=query[b, rows, :].T.astype(BF16),
                xkT=key[b].T.astype(kv_np),
                xvT=value[b].T.astype(kv_np),
                wq=wq_b, wk=wk_b, wv=wv_b, wo=wo_b, bo=bo_f,
            )
        )
    return in_maps


def kernel(query, key, value, Wq, Wk, Wv, Wo, bo):
    runner = _get_runner()
    in_maps = _prep_in_maps(query, key, value, Wq, Wk, Wv, Wo, bo)
    results = runner(in_maps)
    out = np.empty((B, S, D), np.float32)
    for core in range(NCORES):
        b, jj = divmod(core, 2)
        out[b, jj * SH : (jj + 1) * SH, :] = results[core]["outT"].T
    return out


# revision 20
# speedup vs baseline: 9.8482x; 1.5453x over previous
"""Trainium2 Bass kernel for fused multi-head attention block.

Computes (per reference):
    q = query @ Wq ; k = key @ Wk ; v = value @ Wv        (B,S,D)
    per (b,h): A = softmax((q_h @ k_h^T) / sqrt(D)) ; o_h = A @ v_h
    o = merge_heads ; o = q + o
    out = o + relu(o @ Wo + bo)

Shapes: B=4, S=2048, D=1024, H=8 (head dim 128), fp32 in/out.

Sharding: 8 cores = (batch b in 0..3) x (query-seq half j in 0..1).
Each core computes 1024 query rows of one batch against that batch's
full K/V. No collectives. All device compute stays in a
feature-on-partition ("transposed") layout so no on-device transposes
are needed:
  - projections:   lhsT = weight tile (stationary), rhs = x^T     -> q^T, k^T
                   lhsT = xv^T tile,  rhs = Wv                    -> v natural
  - logits^T:      lhsT = k^T tile,   rhs = q^T                   -> [kv, q]
  - E = exp(logits^T / 32) on ScalarE (no max subtraction needed:
    logits/32 ~ N(0, 0.35), exp cannot overflow)
  - o^T:           lhsT = v natural,  rhs = E, accumulated over kv tiles
  - denom:         lhsT = ones[128,.,1], rhs = E  (M=1 matmuls)
  - z^T:           lhsT = Wo tile,    rhs = oq^T

Precision: q path (q projection, logits, z projection, residuals) runs
bf16 matmuls with fp32 PSUM; the attention-internal path (k/v
projections, exp tiles, A@V, denom) runs fp8e4m3 with DoubleRow perf
mode (2 contraction tiles per matmul) — softmax averages ~2048 values
so elementwise fp8 noise washes out in the output.

Schedule: single fused loop. Pass 1 (query chunk c=0) interleaves per
head: q/k projections, v projection tiles, attention. Pass 2 (c=1)
interleaves the c=0 output projection into the exp-bound attention
stream; the c=1 output projection is the only serial tail.
"""

import numpy as np
import ml_dtypes

BF16 = ml_dtypes.bfloat16
FP8 = ml_dtypes.float8_e4m3

FP8_ATTN = True     # exp tiles, A@V, denominator, vN storage in fp8 + DoubleRow
FP8_KVPROJ = True   # k/v projections from fp8 inputs with DoubleRow

B, S, D, H = 4, 2048, 1024, 8
DH = D // H          # 128
SH = S // 2          # 1024 query rows per core
NCORES = 8
KT = D // 128        # 8 contraction tiles
NKV = S // 128       # 16 kv tiles
QC = SH // 512       # 2 query chunks of 512
KC = S // 512        # 4 kv chunks of 512
DC = D // 512        # 2 dim chunks of 512
SCALE = 1.0 / 32.0   # 1/sqrt(D)

_CACHE = {}


def _build_nc():
    import concourse.bacc as bacc
    import concourse.tile as tile
    import concourse.mybir as mybir

    dt = mybir.dt
    AF = mybir.ActivationFunctionType
    ALU = mybir.AluOpType
    DR = mybir.MatmulPerfMode.DoubleRow
    kv_dt = dt.float8e4 if FP8_KVPROJ else dt.bfloat16
    at_dt = dt.float8e4 if FP8_ATTN else dt.bfloat16

    nc = bacc.Bacc("TRN2", target_bir_lowering=False, debug=False)

    xqT = nc.dram_tensor("xqT", [D, SH], dt.bfloat16, kind="ExternalInput").ap()
    xkT = nc.dram_tensor("xkT", [D, S], kv_dt, kind="ExternalInput").ap()
    xvT = nc.dram_tensor("xvT", [D, S], kv_dt, kind="ExternalInput").ap()
    wq = nc.dram_tensor("wq", [D, D], dt.bfloat16, kind="ExternalInput").ap()
    wk = nc.dram_tensor("wk", [D, D], kv_dt, kind="ExternalInput").ap()
    wv = nc.dram_tensor("wv", [D, D], kv_dt, kind="ExternalInput").ap()
    wo = nc.dram_tensor("wo", [D, D], dt.bfloat16, kind="ExternalInput").ap()
    bo = nc.dram_tensor("bo", [128, KT], dt.float32, kind="ExternalInput").ap()
    outT = nc.dram_tensor("outT", [D, SH], dt.float32, kind="ExternalOutput").ap()

    with tile.TileContext(nc) as tc:
        with (
            tc.tile_pool(name="persist", bufs=1) as persist,
            tc.tile_pool(name="xw", bufs=1) as xw,
            tc.tile_pool(name="etp", bufs=8) as etp,
            tc.tile_pool(name="p2t", bufs=2) as p2t,
            tc.tile_pool(name="psproj", bufs=3, space="PSUM") as psproj,
            tc.tile_pool(name="ps_l", bufs=2, space="PSUM") as psl,
            tc.tile_pool(name="ps_o", bufs=1, space="PSUM") as pso,
        ):
            qT = persist.tile([128, H, SH], dt.bfloat16)   # [dh, head, qtok]
            kT = persist.tile([128, H, S], dt.bfloat16)    # [dh, head, kvtok]
            vN = persist.tile([128, NKV, D], at_dt)        # [kvtok%128, kvtile, d]
            oq_b = persist.tile([128, H, SH], dt.bfloat16)
            wo_sb = persist.tile([128, KT, D], dt.bfloat16)
            bo_sb = persist.tile([128, KT], dt.float32)
            ones_sb = persist.tile([128, 2, 16], at_dt)
            nc.gpsimd.memset(ones_sb[:], 1.0)

            xq_sb = xw.tile([128, KT, SH], dt.bfloat16)
            xk_sb = xw.tile([128, KT, S], kv_dt)
            xv_sb = xw.tile([128, KT, S], kv_dt)
            wq_sb = xw.tile([128, KT, D], dt.bfloat16)
            wk_sb = xw.tile([128, KT, D], kv_dt)
            wv_sb = xw.tile([128, KT, D], kv_dt)
            # DMA issue order matches first consumption: q, k weights/inputs
            # (head 0 projections), then v, then the late-used Wo / bo.
            for k in range(KT):
                r = slice(k * 128, (k + 1) * 128)
                nc.sync.dma_start(wq_sb[:, k, :], wq[r, :])
                nc.sync.dma_start(xq_sb[:, k, :], xqT[r, :])
            for k in range(KT):
                r = slice(k * 128, (k + 1) * 128)
                nc.sync.dma_start(wk_sb[:, k, :], wk[r, :])
                nc.sync.dma_start(xk_sb[:, k, :], xkT[r, :])
            for k in range(KT):
                r = slice(k * 128, (k + 1) * 128)
                nc.sync.dma_start(wv_sb[:, k, :], wv[r, :])
                nc.sync.dma_start(xv_sb[:, k, :], xvT[r, :])
            for k in range(KT):
                nc.sync.dma_start(wo_sb[:, k, :], wo[k * 128 : (k + 1) * 128, :])
            nc.sync.dma_start(bo_sb[:], bo[:])

            def q_proj(h):
                hs = slice(h * 128, (h + 1) * 128)
                for cc in range(QC):
                    cs = slice(cc * 512, (cc + 1) * 512)
                    ps = psproj.tile([128, 512], dt.float32, tag="pp", name="pp")
                    for k in range(KT):
                        nc.tensor.matmul(
                            ps[:], wq_sb[:, k, hs], xq_sb[:, k, cs],
                            start=(k == 0), stop=(k == KT - 1),
                        )
                    nc.scalar.copy(qT[:, h, cs], ps[:])

            def k_proj(h):
                hs = slice(h * 128, (h + 1) * 128)
                for cc in range(KC):
                    cs = slice(cc * 512, (cc + 1) * 512)
                    ps = psproj.tile([128, 512], dt.float32, tag="pp", name="pp")
                    if FP8_KVPROJ:
                        for k in range(KT // 2):
                            nc.tensor.matmul(
                                ps[:],
                                wk_sb[:, 2 * k : 2 * k + 2, hs],
                                xk_sb[:, 2 * k : 2 * k + 2, cs],
                                start=(k == 0), stop=(k == KT // 2 - 1),
                                perf_mode=DR,
                            )
                    else:
                        for k in range(KT):
                            nc.tensor.matmul(
                                ps[:], wk_sb[:, k, hs], xk_sb[:, k, cs],
                                start=(k == 0), stop=(k == KT - 1),
                            )
                    nc.vector.tensor_copy(kT[:, h, cs], ps[:])

            def v_proj(t, dc):
                ts_ = slice(t * 128, (t + 1) * 128)
                cs = slice(dc * 512, (dc + 1) * 512)
                ps = psproj.tile([128, 512], dt.float32, tag="pp", name="pp")
                if FP8_KVPROJ:
                    for k in range(KT // 2):
                        nc.tensor.matmul(
                            ps[:],
                            xv_sb[:, 2 * k : 2 * k + 2, ts_],
                            wv_sb[:, 2 * k : 2 * k + 2, cs],
                            start=(k == 0), stop=(k == KT // 2 - 1),
                            perf_mode=DR,
                        )
                else:
                    for k in range(KT):
                        nc.tensor.matmul(
                            ps[:], xv_sb[:, k, ts_], wv_sb[:, k, cs],
                            start=(k == 0), stop=(k == KT - 1),
                        )
                nc.vector.tensor_copy(vN[:, t, cs], ps[:])

            def attn(h, c):
                hs = slice(h * 128, (h + 1) * 128)
                cs = slice(c * 512, (c + 1) * 512)
                ets = []
                for tp in range(NKV // 2):
                    # two logits matmuls into one 2-bank PSUM tile, then a
                    # single wide exp on ScalarE
                    pl = psl.tile([128, 1024], dt.float32, tag="pl", name="pl")
                    for u in range(2):
                        t = 2 * tp + u
                        ts_ = slice(t * 128, (t + 1) * 128)
                        nc.tensor.matmul(
                            pl[:, u * 512 : (u + 1) * 512],
                            kT[:, h, ts_], qT[:, h, cs],
                        )
                    et = etp.tile([128, 1024], at_dt, tag="et", name="et")
                    nc.scalar.activation(et[:], pl[:], AF.Exp, scale=SCALE)
                    ets.append(et)
                po = pso.tile([128, 512], dt.float32, tag="po", name="po")
                pdt = psproj.tile([128, 512], dt.float32, tag="pp", name="pdt")
                pd = pdt[0:1, :]
                if FP8_ATTN:
                    for tp in range(NKV // 2):
                        pair = ets[tp][:].rearrange("p (u n) -> p u n", u=2)
                        nc.tensor.matmul(
                            po[:], vN[:, 2 * tp : 2 * tp + 2, hs], pair,
                            start=(tp == 0), stop=(tp == NKV // 2 - 1),
                            perf_mode=DR,
                        )
                    for tp in range(NKV // 2):
                        pair = ets[tp][:].rearrange("p (u n) -> p u n", u=2)
                        nc.tensor.matmul(
                            pd, ones_sb[:, :, 0:1], pair,
                            start=(tp == 0), stop=(tp == NKV // 2 - 1),
                            perf_mode=DR,
                        )
                else:
                    for tp in range(NKV // 2):
                        for u in range(2):
                            t = 2 * tp + u
                            us = slice(u * 512, (u + 1) * 512)
                            nc.tensor.matmul(
                                po[:], vN[:, t, hs], ets[tp][:, us],
                                start=(t == 0), stop=(t == NKV - 1),
                            )
                    for tp in range(NKV // 2):
                        for u in range(2):
                            t = 2 * tp + u
                            us = slice(u * 512, (u + 1) * 512)
                            nc.tensor.matmul(
                                pd, ones_sb[:, 0, 0:1], ets[tp][:, us],
                                start=(t == 0), stop=(t == NKV - 1),
                            )
                dn = p2t.tile([1, 512], dt.float32, tag="dn", name="dn")
                nc.vector.tensor_copy(dn[:], pd)
                rb = p2t.tile([128, 512], dt.float32, tag="rb", name="rb")
                nc.gpsimd.partition_broadcast(rb[:], dn[:])
                nc.vector.reciprocal(rb[:], rb[:])
                on = p2t.tile([128, 512], dt.float32, tag="on", name="on")
                nc.vector.tensor_mul(on[:], po[:], rb[:])
                nc.vector.tensor_add(oq_b[:, h, cs], on[:], qT[:, h, cs])

            def z_group(nt, c, tail=False):
                ns = slice(nt * 128, (nt + 1) * 128)
                cs = slice(c * 512, (c + 1) * 512)
                pz = psproj.tile([128, 512], dt.float32, tag="pp", name="pp")
                for k in range(KT):
                    nc.tensor.matmul(
                        pz[:], wo_sb[:, k, ns], oq_b[:, k, cs],
                        start=(k == 0), stop=(k == KT - 1),
                    )
                rel = p2t.tile([128, 512], dt.float32, tag="rel", name="rel")
                if tail:
                    nc.scalar.activation(
                        rel[:], pz[:], AF.Relu, bias=bo_sb[:, nt : nt + 1]
                    )
                else:
                    nc.vector.tensor_scalar(
                        rel[:], pz[:], bo_sb[:, nt : nt + 1], 0.0, ALU.add, ALU.max
                    )
                ot = p2t.tile([128, 512], dt.float32, tag="ot", name="ot")
                nc.vector.tensor_add(ot[:], rel[:], oq_b[:, nt, cs])
                nc.sync.dma_start(outT[ns, cs], ot[:])

            # ---- pass 1: projections + attention for query chunk c=0
            for h in range(H):
                q_proj(h)
                k_proj(h)
                if h == 0:
                    for t in range(NKV):
                        v_proj(t, 0)
                attn(h, 0)
                if h < 4:
                    for t in range(4 * h, 4 * h + 4):
                        v_proj(t, 1)
            # ---- pass 2: attention c=1 with c=0 output projection interleaved
            for h in range(H):
                attn(h, 1)
                z_group(h, 0)
            # ---- tail: c=1 output projection
            for nt in range(KT):
                z_group(nt, 1, tail=True)

    nc.compile()
    return nc


def _make_runner(nc, n_cores=NCORES):
    import jax
    from jax.experimental.shard_map import shard_map
    from jax.sharding import Mesh, PartitionSpec
    from concourse import bass2jax
    import concourse.mybir as mybir

    bass2jax.install_neuronx_cc_hook()
    assert nc.dbg_addr is None

    partition_name = nc.partition_id_tensor.name if nc.partition_id_tensor else None
    in_names, out_names, out_avals, zero_outs = [], [], [], []
    for alloc in nc.m.functions[0].allocations:
        if not isinstance(alloc, mybir.MemoryLocationSet):
            continue
        name = alloc.memorylocations[0].name
        if alloc.kind == "ExternalInput":
            if name != partition_name:
                in_names.append(name)
        elif alloc.kind == "ExternalOutput":
            shape = tuple(alloc.tensor_shape)
            dtype = mybir.dt.np(alloc.dtype)
            out_names.append(name)
            out_avals.append(jax.core.ShapedArray(shape, dtype))
            zero_outs.append(np.zeros(shape, dtype))
    n_params = len(in_names)
    n_outs = len(out_avals)
    in_names_all = list(in_names) + list(out_names)
    if partition_name is not None:
        in_names_all.append(partition_name)
    donate = tuple(range(n_params, n_params + n_outs))

    def _body(*args):
        operands = list(args)
        if partition_name is not None:
            operands.append(bass2jax.partition_id_tensor())
        outs = bass2jax._bass_exec_p.bind(
            *operands,
            out_avals=tuple(out_avals),
            in_names=tuple(in_names_all),
            out_names=tuple(out_names),
            lowering_input_output_aliases=(),
            sim_require_finite=True,
            sim_require_nnan=True,
            nc=nc,
        )
        return tuple(outs)

    devices = jax.devices()[:n_cores]
    assert len(devices) == n_cores
    mesh = Mesh(np.asarray(devices), ("core",))
    in_specs = (PartitionSpec("core"),) * (n_params + n_outs)
    out_specs = (PartitionSpec("core"),) * n_outs
    sharded = jax.jit(
        shard_map(
            _body, mesh=mesh, in_specs=in_specs, out_specs=out_specs, check_rep=False
        ),
        donate_argnums=donate,
        keep_unused=True,
    )

    class Runner:
        def __init__(self):
            self.mesh = mesh
            self.in_names = in_names
            self.out_names = out_names
            self.zero_outs = zero_outs
            self.n_cores = n_cores
            self.sharded = sharded

        def concat_inputs(self, in_maps):
            per_core = [[np.asarray(m[nm]) for nm in in_names] for m in in_maps]
            return [
                np.concatenate([per_core[c][i] for c in range(n_cores)], axis=0)
                for i in range(n_params)
            ]

        def concat_zeros(self):
            return [np.concatenate([z] * n_cores, axis=0) for z in zero_outs]

        def __call__(self, in_maps):
            concat_in = self.concat_inputs(in_maps)
            out_arrs = sharded(*concat_in, *self.concat_zeros())
            results = []
            for c in range(n_cores):
                res = {}
                for i, name in enumerate(out_names):
                    arr = np.asarray(out_arrs[i])
                    rows = arr.shape[0] // n_cores
                    res[name] = arr[c * rows : (c + 1) * rows]
                results.append(res)
            return results

    return Runner()


def _get_runner():
    if "runner" not in _CACHE:
        nc = _build_nc()
        _CACHE["runner"] = _make_runner(nc)
    return _CACHE["runner"]


def _prep_in_maps(query, key, value, Wq, Wk, Wv, Wo, bo):
    kv_np = FP8 if FP8_KVPROJ else BF16
    query = np.asarray(query, np.float32)
    key = np.asarray(key, np.float32)
    value = np.asarray(value, np.float32)
    wq_b = np.asarray(Wq, np.float32).astype(BF16)
    wk_b = np.asarray(Wk, np.float32).astype(kv_np)
    wv_b = np.asarray(Wv, np.float32).astype(kv_np)
    wo_b = np.asarray(Wo, np.float32).astype(BF16)
    bo_f = np.ascontiguousarray(np.asarray(bo, np.float32).reshape(KT, 128).T)
    in_maps = []
    for core in range(NCORES):
        b, jj = divmod(core, 2)
        rows = slice(jj * SH, (jj + 1) * SH)
        in_maps.append(
            dict(
                xqT=query[b, rows, :].T.astype(BF16),
                xkT=key[b].T.astype(kv_np),
                xvT=value[b].T.astype(kv_np),
                wq=wq_b, wk=wk_b, wv=wv_b, wo=wo_b, bo=bo_f,
            )
        )
    return in_maps


def kernel(query, key, value, Wq, Wk, Wv, Wo, bo):
    runner = _get_runner()
    in_maps = _prep_in_maps(query, key, value, Wq, Wk, Wv, Wo, bo)
    results = runner(in_maps)
    out = np.empty((B, S, D), np.float32)
    for core in range(NCORES):
        b, jj = divmod(core, 2)
        out[b, jj * SH : (jj + 1) * SH, :] = results[core]["outT"].T
    return out
